# revision 1
# baseline (speedup 1.0000x reference)
"""Correlation kernel (max_disp=1, 9 offsets) for Trainium2, 8 NeuronCores.

Computation (per batch b):
    out[dx*3+dy, i, j] = mean_c( x1[c,i,j] * pad(x2)[c, i+dy, j+dx] )
with B=8, C=512, H=W=96, pad=1 on each spatial side.

Sharding: data-parallel over batch — core b handles batch b. No collectives.

Per-core strategy (v3):
  - C (512) on SBUF partitions, 4 chunks of 128; spatial (96x96=9216) on free dim.
  - Inputs DMA'd with fp32->bf16 cast (SWDGE, gpsimd triggers), split into
    row-halves so compute starts after ~half the first chunk's bytes land.
  - x2 lands in a zero-padded [128, 98, 98] tile; two flat-shifted copies
    (od0 rows 0..49, od1 rows 48..97; SBUF->SBUF DMA on the SP HWDGE ring)
    keep dx=1 views 4B-aligned so DVE tensor_mul runs in 2x mode.
  - Products are computed per row-half ([128, 48, 96] bf16, 72 of them),
    split between DVE (2x mode) and GpSimd/Pool (slow but otherwise idle).
    Pool gets dx!=1 offsets only (no od dependency); DVE does od-based
    offsets last.
  - TensorE reduces over partitions via matmuls with a 32-wide sliding
    one-hot-column stationary (LDWEIGHTS = 32 cols): offset k, global block
    bb (= 9*half + blk) -> row m = 18k + bb -> PSUM bank m//32, row m%32.
    Six [32, 512] PSUM tiles (one bank each); every MM writes the full tile
    so one start=True per bank is safe with bank-granular has_written.
  - Act engine scales PSUM banks (in completion order) by 1/512 into SBUF
    and issues the output DMAs.
"""

import os
import sys

for _p in ("/opt/trn_rl_repo",):
    if os.path.isdir(_p) and _p not in sys.path:
        sys.path.insert(0, _p)

from contextlib import ExitStack

import numpy as np

import concourse.bass as bass
import concourse.mybir as mybir
import concourse.tile as tile
from concourse import bacc
from concourse.bass_utils import run_bass_kernel_spmd

B, C, H, W = 8, 512, 96, 96
NCORES = 8
PW = W + 2          # padded spatial width
HH = H // 2         # 48 rows per half
NCHUNK = C // 128   # 4
NBLK = HH * W // 512  # 9 512-col blocks per half
F32 = mybir.dt.float32
BF16 = mybir.dt.bfloat16


# (ch, k) product tiles computed on GpSimd (Pool) instead of DVE. Only dx!=1
# offsets (k not in 3..5) so Pool never waits on the od copies.
def _pool_set():
    spec = os.environ.get("CORR_POOL_SET")
    if spec:
        if not spec.strip():
            return set()
        return {tuple(map(int, p.split(":"))) for p in spec.split(",")}
    n = int(os.environ.get("CORR_POOL_MULS", "5"))
    # Weighted toward later chunks: the serial HBM feed (~26.4 us/chunk) lags
    # compute early on, so early chunks keep more work on the (faster) DVE.
    cand = [(0, 8), (1, 0), (2, 0), (2, 8), (3, 0), (1, 8), (3, 8), (0, 0)]
    return set(cand[:n])


def _corr_body(ctx: ExitStack, tc: "tile.TileContext", out_t, x1_t, x2_t, nchunk=NCHUNK):
    nc = tc.nc
    pool_set = _pool_set()

    wpool = ctx.enter_context(tc.tile_pool(name="wm", bufs=1))
    x1pool = ctx.enter_context(tc.tile_pool(name="x1", bufs=2))
    evpool = ctx.enter_context(tc.tile_pool(name="ev", bufs=3))
    odpool = ctx.enter_context(tc.tile_pool(name="od", bufs=2))
    prpool = ctx.enter_context(tc.tile_pool(name="pr", bufs=int(os.environ.get("CORR_PROD_BUFS", "5"))))
    pppool = ctx.enter_context(tc.tile_pool(name="pp", bufs=int(os.environ.get("CORR_PPROD_BUFS", "2"))))
    pspool = ctx.enter_context(
        tc.tile_pool(name="ps", bufs=1, space=bass.MemorySpace.PSUM)
    )
    outpool = ctx.enter_context(tc.tile_pool(name="ot", bufs=1))

    # Sliding 32-wide one-hot stationaries: slice [:, s : s+32] has its all-ones
    # column at local position r when s = ones_col - r. Two masters (ones at
    # cols 30 and 31) keep s even for either parity of r, so every LDWEIGHTS
    # source is 4-byte aligned (bf16).
    wmE = wpool.tile([128, 64], BF16)
    nc.vector.memset(wmE[:, :], 0.0)
    nc.vector.memset(wmE[:, 30:31], 1.0)
    wmO = wpool.tile([128, 64], BF16)
    nc.vector.memset(wmO[:, :], 0.0)
    nc.vector.memset(wmO[:, 31:32], 1.0)

    def wslice(r: int):
        wm, col = (wmE, 30) if r % 2 == 0 else (wmO, 31)
        s = col - r
        return wm[:, s : s + 32]

    # 6 PSUM accumulators, one bank each, 32 rows used per bank (162 rows
    # total). Every MM writes the full [32, 512] tile (one-hot row gets the
    # sum, the rest accumulate zeros), so a single start=True per bank works
    # with the bank-granular has_written clear.
    ps = [pspool.tile([32, 512], F32, name=f"ps{t}") for t in range(6)]

    x1f = x1_t.ap()  # [512, 96, 96] f32 DRAM
    x2f = x2_t.ap()

    def ev_ks(ch):
        return [k for k in (0, 1, 2, 6, 7, 8) if (ch, k) not in pool_set]

    def pool_ks(ch):
        return [k for k in (0, 1, 2, 6, 7, 8) if (ch, k) in pool_set]

    OD_KS = [3, 4, 5]

    # Per-chunk phase sequence: ("dve", k, row0, nrows) products (+ their MMs)
    # and ("pool_mm", k, h) MM groups for Pool-computed half products. Chunk 0
    # runs its ev-based products in thirds (32 rows) so compute starts as soon
    # as the first third of the first chunk's bytes lands.
    def phases(ch):
        if ch == 0:
            return (
                [("dve", k, 0, 32) for k in ev_ks(ch)]
                + [("dve", k, 32, 32) for k in ev_ks(ch)]
                + [("dve", k, 0, 48) for k in OD_KS]
                + [("pool_mm", k, 0) for k in pool_ks(ch)]
                + [("dve", k, 64, 32) for k in ev_ks(ch)]
                + [("dve", k, 48, 48) for k in OD_KS]
                + [("pool_mm", k, 1) for k in pool_ks(ch)]
            )
        return (
            [("dve", k, 0, 48) for k in ev_ks(ch)]
            + [("dve", k, 0, 48) for k in OD_KS]
            + [("pool_mm", k, 0) for k in pool_ks(ch)]
            + [("dve", k, 48, 48) for k in ev_ks(ch)]
            + [("dve", k, 48, 48) for k in OD_KS]
            + [("pool_mm", k, 1) for k in pool_ks(ch)]
        )

    # Emission-order plan of all MMs: (ch, k, gb) with gb the global 512-col
    # block index (row0*3//16 + j); PSUM row m = 18k + gb, bank q = m//32.
    mm_plan = []
    for ch in range(nchunk):
        for item in phases(ch):
            if item[0] == "dve":
                _, k, row0, nrows = item
            else:
                _, k, h = item
                row0, nrows = 48 * h, 48
            for j in range(nrows * 3 // 16):
                mm_plan.append((ch, k, row0 * 3 // 16 + j))
    last_mm_for_bank = {}
    bank_completion = []
    for ch, k, gb in mm_plan:
        q = (18 * k + gb) // 32
        last_mm_for_bank[q] = (ch, k, gb)
        if q in bank_completion:
            bank_completion.remove(q)
        bank_completion.append(q)

    started = [False] * 6

    x1bf = [None] * nchunk
    ev = [None] * nchunk
    od0 = [None] * nchunk
    od1 = [None] * nchunk

    def emit_ev_tile(ch):
        t = evpool.tile([128, PW, PW], BF16, name="ev")
        ev[ch] = t
        # borders on the gpsimd stream (cheap; WAR-free with ev bufs=3)
        nc.gpsimd.memset(t[:, 0, :], 0.0)
        nc.gpsimd.memset(t[:, PW - 1, :], 0.0)
        nc.gpsimd.memset(t[:, 1 : PW - 1, 0], 0.0)
        nc.gpsimd.memset(t[:, 1 : PW - 1, PW - 1], 0.0)
        return t

    def emit_ev_dma(ch, r0, r1):
        p0 = ch * 128
        nc.gpsimd.dma_start(
            out=ev[ch][:, 1 + r0 : 1 + r1, 1 : PW - 1],
            in_=x2f[p0 : p0 + 128, r0:r1, :],
        )

    def emit_x1_tile(ch):
        x1bf[ch] = x1pool.tile([128, H, W], BF16, name="x1bf")

    def emit_x1_dma(ch, r0, r1):
        p0 = ch * 128
        nc.gpsimd.dma_start(
            out=x1bf[ch][:, r0:r1, :], in_=x1f[p0 : p0 + 128, r0:r1, :]
        )

    def emit_od_copies(ch):
        ev_flat = ev[ch][:, :, :].rearrange("p a b -> p (a b)")
        # odd copies: flat shift-by-one so dx=1 views are 4B-aligned for the
        # DVE 2x mode. Copied on the otherwise-idle Act engine (alignment-
        # agnostic, no DMA-queue or SBUF-fabric contention).
        # od0 covers padded rows 0..49, od1 rows 48..97 (2-row overlap).
        o0 = odpool.tile([128, 50, PW], BF16, name="od0")
        od0[ch] = o0
        o0_flat = o0[:, :, :].rearrange("p a b -> p (a b)")
        nc.scalar.copy(o0_flat[:, 0 : 50 * PW], ev_flat[:, 1 : 50 * PW + 1])
        o1 = odpool.tile([128, 50, PW], BF16, name="od1")
        od1[ch] = o1
        o1_flat = o1[:, :, :].rearrange("p a b -> p (a b)")
        nc.scalar.copy(
            o1_flat[:, 0 : 50 * PW - 1], ev_flat[:, 48 * PW + 1 : PW * PW]
        )

    def emit_loads_head(ch, cuts):
        # interleaved ev/x1 loads for the first two chunks (head latency)
        emit_ev_tile(ch)
        emit_x1_tile(ch)
        evcuts = [0] + [c + 2 for c in cuts[1:-1]] + [H]  # ev needs 2 extra rows
        for i in range(len(cuts) - 1):
            emit_ev_dma(ch, evcuts[i], evcuts[i + 1])
            emit_x1_dma(ch, cuts[i], cuts[i + 1])
        emit_od_copies(ch)

    def emit_ev_loads(ch):
        emit_ev_tile(ch)
        emit_ev_dma(ch, 0, 50)
        emit_ev_dma(ch, 50, H)
        emit_od_copies(ch)

    def emit_x1_loads(ch):
        emit_x1_tile(ch)
        emit_x1_dma(ch, 0, HH)
        emit_x1_dma(ch, HH, H)

    def view_for(ch, k, row0, nrows):
        dx, dy = k // 3, k % 3
        if dx == 1:
            assert (row0, nrows) in ((0, 48), (48, 48))
            src = od0[ch] if row0 == 0 else od1[ch]
            return src[:, dy : dy + 48, 0:W]
        return ev[ch][:, row0 + dy : row0 + dy + nrows, dx : dx + W]

    def emit_mms(ch, k, row0, nrows, prod):
        prod_flat = prod[:, :, :].rearrange("p a b -> p (a b)")
        gb0 = row0 * 3 // 16
        for j in range(nrows * 3 // 16):
            gb = gb0 + j
            m = 18 * k + gb
            q, r = m // 32, m % 32
            st = not started[q]
            started[q] = True
            last = last_mm_for_bank[q] == (ch, k, gb)
            nc.tensor.matmul(
                ps[q][:, :],
                wslice(r),
                prod_flat[:, j * 512 : (j + 1) * 512],
                start=st,
                stop=last,
            )

    emit_loads_head(0, [0, 32, 64, H])
    if nchunk > 1:
        emit_loads_head(1, [0, HH, H])

    for ch in range(nchunk):
        # ev loads for chunk ch+2: WAR-free with ev bufs=3, so the triggers
        # fire immediately and never block the Pool products queued behind.
        if ch + 2 < nchunk:
            emit_ev_loads(ch + 2)
        # Pool products for this chunk (long-running; start early).
        prods = {}
        for h in range(2):
            for k in pool_ks(ch):
                prod = pppool.tile([128, HH, W], BF16, name="pprod")
                prods[(k, h)] = prod
                nc.gpsimd.tensor_mul(
                    prod[:, :, :],
                    x1bf[ch][:, 48 * h : 48 * h + HH, :],
                    view_for(ch, k, 48 * h, 48),
                )
        # x1 loads for ch+2 after this chunk's Pool products: the WAR wait
        # (x1 bufs=2, readers = chunk ch's products) sits behind them on the
        # Q7 stream, so it cannot deadlock and delays nothing urgent.
        if ch + 2 < nchunk:
            emit_x1_loads(ch + 2)
        for item in phases(ch):
            if item[0] == "dve":
                _, k, row0, nrows = item
                prod = prpool.tile([128, nrows, W], BF16, name="prod")
                nc.vector.tensor_mul(
                    prod[:, :, :],
                    x1bf[ch][:, row0 : row0 + nrows, :],
                    view_for(ch, k, row0, nrows),
                )
                emit_mms(ch, k, row0, nrows, prod)
            else:
                _, k, h = item
                emit_mms(ch, k, 48 * h, 48, prods[(k, h)])

    outT = [outpool.tile([32, 512], F32, name=f"outT{t}") for t in range(6)]
    for q in bank_completion:
        nc.scalar.mul(outT[q][:, :], ps[q][:, :], 1.0 / (128 * nchunk))

    outf = out_t.ap()  # [9, 96, 96] f32 DRAM
    out_flat = outf.rearrange("k a b -> k (a b)")
    for k in range(9):
        # rows 18k..18k+17 may span two banks; DMA each segment.
        m0 = 18 * k
        seg_start = 0
        while seg_start < 18:
            m = m0 + seg_start
            q, r = m // 32, m % 32
            cnt = min(18 - seg_start, 32 - r)
            nc.sync.dma_start(
                out=out_flat[k, seg_start * 512 : (seg_start + cnt) * 512],
                in_=outT[q][r : r + cnt, :],
            )
            seg_start += cnt


_CACHE = {}


def _build(c=C, debug=False):
    key = ("nc", c, os.environ.get("CORR_POOL_MULS", "7"))
    if key in _CACHE:
        return _CACHE[key]
    nchunk = c // 128
    nc = bacc.Bacc("TRN2", target_bir_lowering=False, debug=debug)
    x1_t = nc.dram_tensor("x_1", [c, H, W], F32, kind="ExternalInput")
    x2_t = nc.dram_tensor("x_2", [c, H, W], F32, kind="ExternalInput")
    out_t = nc.dram_tensor("out", [9, H, W], F32, kind="ExternalOutput")
    with tile.TileContext(nc) as tc, ExitStack() as ctx:
        _corr_body(ctx, tc, out_t, x1_t, x2_t, nchunk=nchunk)
    nc.compile()
    _CACHE[key] = nc
    return nc


def kernel(x_1: np.ndarray, x_2: np.ndarray) -> np.ndarray:
    x_1 = np.ascontiguousarray(np.asarray(x_1), dtype=np.float32)
    x_2 = np.ascontiguousarray(np.asarray(x_2), dtype=np.float32)
    assert x_1.shape == (B, C, H, W) and x_2.shape == (B, C, H, W)
    nc = _build()
    in_maps = [
        {"x_1": x_1[i].copy(), "x_2": x_2[i].copy()} for i in range(NCORES)
    ]
    last_err = None
    for attempt in range(3):
        try:
            res = run_bass_kernel_spmd(nc, in_maps, list(range(NCORES)))
            out = np.stack([res.results[i]["out"] for i in range(NCORES)], axis=0)
            return out.astype(np.float32)
        except Exception as e:  # rare transient device faults — retry
            last_err = e
            import time as _time

            _time.sleep(5.0 * (attempt + 1))
    raise last_err


if __name__ == "__main__":
    rng = np.random.default_rng(0)
    a = rng.standard_normal((B, C, H, W), dtype=np.float32)
    b = rng.standard_normal((B, C, H, W), dtype=np.float32)
    o = kernel(a, b)
    print("out", o.shape, o.dtype, float(np.abs(o).max()))



# revision 2
# speedup vs baseline: 1.4519x; 1.4519x over previous
"""Correlation kernel (max_disp=1, 9 offsets) for Trainium2, 8 NeuronCores.

Computation (per batch b):
    out[dx*3+dy, i, j] = mean_c( x1[c,i,j] * pad(x2)[c, i+dy, j+dx] )
with B=8, C=512, H=W=96, pad=1 on each spatial side.

Sharding: data-parallel over batch - core b handles batch b. No collectives.

Per-core strategy (v4, TensorE band-matmul):
  - Host casts inputs to bf16 (tolerance is 2e-2; bf16 dot error ~5e-3),
    pre-pads x2 to [4ct, 128c, 98, 98], and pre-tiles x1 patch-major as
    [6 pair, 4 ct, 128 c, 12 q, 128 m]  (patch = 8x16 interior pixels,
    m = r*16+s; q = halfbi*6+bj; pair of bi-rows per DMA strip).
  - Device: for each 8x16 interior patch, LDWEIGHTS x1-patch (lhsT
    [c=128, m=128]) and matmul against the 10x18 halo of padded x2
    (rhs [c=128, 180]), accumulating over the 4 c-tiles in PSUM.
    psum[m, n] = sum_c x1[c, pix m] * x2p[c, halo pix n]: all 9 offsets
    of every pixel live on a (partition, free) band of this tile.
  - ScalarE scales psum by 1/512 into a bf16 band tile; band DMAs back to
    DRAM (ACT HWDGE ring so input DMAs on the SP ring never stall).
  - Host extracts the 9 band diagonals (pure gather, no arithmetic
    beyond the device-computed means) into [9, 96, 96].
  - Inputs stream in 6 row-pair strips (x1 1.6MB + x2 ~1.6MB each) so PE
    chases the DMA; everything stays resident in SBUF (~150KB/partition).
"""

import os
import sys

for _p in ("/opt/trn_rl_repo",):
    if os.path.isdir(_p) and _p not in sys.path:
        sys.path.insert(0, _p)

from contextlib import ExitStack

import numpy as np
import ml_dtypes

import concourse.bass as bass
import concourse.mybir as mybir
import concourse.tile as tile
from concourse import bacc
from concourse.bass_utils import run_bass_kernel_spmd

B, C, H, W = 8, 512, 96, 96
NCORES = 8
NCT = C // 128           # 4 channel tiles
PH, PW = H + 2, W + 2    # 98x98 padded x2
PR, PC = 8, 16           # interior patch rows x cols (M = 128 pixels)
HR, HC = PR + 2, PC + 2  # halo 10x18 (N = 180)
NBI, NBJ = H // PR, W // PC   # 12 x 6 patch grid
NPAIR = NBI // 2         # 6 strip groups (2 bi-rows each)
PPG = 2 * NBJ            # patches per strip group = 12
MPIX = PR * PC           # 128
NHALO = HR * HC          # 180
F32 = mybir.dt.float32
BF16 = mybir.dt.bfloat16
BF16NP = ml_dtypes.bfloat16


def _corr_body(ctx: ExitStack, tc: "tile.TileContext", out_t, x1_t, x2_t):
    nc = tc.nc

    x1pool = ctx.enter_context(tc.tile_pool(name="x1", bufs=1))
    x2pool = ctx.enter_context(tc.tile_pool(name="x2", bufs=1))
    bdpool = ctx.enter_context(tc.tile_pool(name="bd", bufs=3))
    pspool = ctx.enter_context(
        tc.tile_pool(name="ps", bufs=8, space=bass.MemorySpace.PSUM)
    )

    SLAB = NCT * PPG * MPIX  # x1 elems per strip group = 6144
    x1t = x1pool.tile([128, NPAIR * SLAB], BF16)
    x2t = x2pool.tile([128, NCT, PH, PW], BF16)

    x1f = x1_t.ap()  # [NPAIR, NCT, 128, PPG*MPIX] bf16 DRAM
    x2f = x2_t.ap()  # [NCT, 128, PH, PW] bf16 DRAM
    outf = out_t.ap()  # [NPAIR, 128, PPG, NHALO] bf16 DRAM

    # Input strips, interleaved x1/x2 by row-pair group so PE can chase.
    # x2 strip b covers padded rows row0[b]..row0[b+1] (18 rows, then 16).
    row0 = [0] + [16 * b + 18 for b in range(NPAIR)]
    for b in range(NPAIR):
        base = b * SLAB
        nc.sync.dma_start(
            out=x1t[:, base : base + SLAB].rearrange("c (t f) -> c t f", t=NCT),
            in_=x1f[b].rearrange("t c f -> c t f"),
        )
        r0, r1 = row0[b], row0[b + 1]
        nc.sync.dma_start(
            out=x2t[:, :, r0:r1, :],
            in_=x2f[:, :, r0:r1, :].rearrange("t c h w -> c t h w"),
        )

    inv = 1.0 / C
    for b in range(NPAIR):
        band = bdpool.tile([128, PPG, NHALO], BF16, name="band")
        for q in range(PPG):
            halfbi, bj = divmod(q, NBJ)
            bi = 2 * b + halfbi
            ps = pspool.tile([MPIX, NHALO], F32, name="ps")
            for t in range(NCT):
                lbase = ((b * NCT + t) * PPG + q) * MPIX
                nc.tensor.matmul(
                    ps[:, :],
                    x1t[:, lbase : lbase + MPIX],
                    x2t[:, t, PR * bi : PR * bi + HR, PC * bj : PC * bj + HC],
                    start=(t == 0),
                    stop=(t == NCT - 1),
                )
            nc.scalar.mul(band[:, q, :], ps[:, :], inv)
        # ACT HWDGE ring: naturally ordered after this band's ScalarE muls,
        # never stalls the SP input-DMA ring.
        nc.scalar.dma_start(out=outf[b], in_=band[:, :, :])


_CACHE = {}


def _build(debug=False):
    key = "nc"
    if key in _CACHE:
        return _CACHE[key]
    nc = bacc.Bacc("TRN2", target_bir_lowering=False, debug=debug)
    x1_t = nc.dram_tensor(
        "x1w", [NPAIR, NCT, 128, PPG * MPIX], BF16, kind="ExternalInput"
    )
    x2_t = nc.dram_tensor("x2p", [NCT, 128, PH, PW], BF16, kind="ExternalInput")
    out_t = nc.dram_tensor(
        "out", [NPAIR, 128, PPG, NHALO], BF16, kind="ExternalOutput"
    )
    with tile.TileContext(nc) as tc, ExitStack() as ctx:
        _corr_body(ctx, tc, out_t, x1_t, x2_t)
    nc.compile()
    _CACHE[key] = nc
    return nc


def prep_core_inputs(x1b: np.ndarray, x2b: np.ndarray) -> dict:
    """Pack one sample's fp32 [C,H,W] pair into the device layouts."""
    a = x1b.astype(BF16NP).reshape(NCT, 128, NPAIR, 2, PR, NBJ, PC)
    # -> [pair, ct, c, halfbi, bj, r, s]
    a = np.ascontiguousarray(a.transpose(2, 0, 1, 3, 5, 4, 6))
    x1w = a.reshape(NPAIR, NCT, 128, PPG * MPIX)
    x2p = np.zeros((NCT, 128, PH, PW), BF16NP)
    x2p[:, :, 1 : H + 1, 1 : W + 1] = x2b.astype(BF16NP).reshape(NCT, 128, H, W)
    return {"x1w": x1w, "x2p": x2p}


_RR, _SS = np.mgrid[0:PR, 0:PC]


def extract_band(band: np.ndarray) -> np.ndarray:
    """[NPAIR, 128, PPG, NHALO] band (already scaled by 1/C) -> [9, H, W]."""
    v = np.asarray(band).astype(np.float32)
    v = v.reshape(NPAIR, PR, PC, 2, NBJ, NHALO)  # [pair, r, s, halfbi, bj, n]
    out = np.empty((9, H, W), np.float32)
    for dx in range(3):
        for dy in range(3):
            n = HC * (_RR + dy) + _SS + dx  # [PR, PC]
            g = v[:, _RR, _SS, :, :, n]  # -> [r, s, pair, halfbi, bj]
            out[3 * dx + dy] = (
                g.transpose(2, 3, 0, 4, 1).reshape(H, W)
            )
    return out


def kernel(x_1: np.ndarray, x_2: np.ndarray) -> np.ndarray:
    x_1 = np.ascontiguousarray(np.asarray(x_1), dtype=np.float32)
    x_2 = np.ascontiguousarray(np.asarray(x_2), dtype=np.float32)
    assert x_1.shape == (B, C, H, W) and x_2.shape == (B, C, H, W)
    nc = _build()
    in_maps = [prep_core_inputs(x_1[i], x_2[i]) for i in range(NCORES)]
    last_err = None
    for attempt in range(3):
        try:
            res = run_bass_kernel_spmd(nc, in_maps, list(range(NCORES)))
            out = np.stack(
                [extract_band(res.results[i]["out"]) for i in range(NCORES)],
                axis=0,
            )
            return out.astype(np.float32)
        except Exception as e:  # rare transient device faults - retry
            last_err = e
            import time as _time

            _time.sleep(5.0 * (attempt + 1))
    raise last_err


if __name__ == "__main__":
    rng = np.random.default_rng(0)
    a = rng.standard_normal((B, C, H, W), dtype=np.float32)
    b = rng.standard_normal((B, C, H, W), dtype=np.float32)
    o = kernel(a, b)
    print("out", o.shape, o.dtype, float(np.abs(o).max()))
